# revision 21
# baseline (speedup 1.0000x reference)
"""Trainium2 Bass kernel for nn_DeformGCN (6-layer GCN + dense decoder).

Strategy:
  - Host precompute from `edges`: dense propagation matrix P (N x N) with
    P[dst,src] += 1/sqrt(deg_s * deg_d) and P[n,n] += 1/deg_n, then P2 = P @ P.
    GCN layer pairs (no nonlinearity between even->odd layers) fuse into
    3 stages:  z = P2 @ (h @ (Wa@Wb)) + outer(r, ba@Wb) + bb,  r = P @ 1,
    followed by LeakyReLU(0.01).
  - Data-parallel over batch: each of 8 cores runs the GCN on 2 batches.
  - feat = h5.reshape(B, 6144) is AllGather'd across cores; the 6144x6144
    decoder weight is column-sharded (768 cols/core): y_c = tanh(feat @ Wd_c
    + bd_c) * 0.1.
  - All matmuls in bf16 with f32 PSUM accumulation.
"""

import numpy as np
import ml_dtypes

import bass_rust
import concourse.bass as bass
import concourse.mybir as mybir
import concourse.tile as tile
from concourse.tile import ScopedClock
from concourse.bass_utils import run_bass_kernel_spmd

BF16 = mybir.dt.bfloat16
F32 = mybir.dt.float32
NPBF16 = ml_dtypes.bfloat16

N_CORES = 8
B = 16
N = 2048
C_IN = 1475
C_PAD = 1536  # 12 * 128
NT = N // 128          # 16 node tiles
CT = C_PAD // 128      # 12 channel tiles
BL = B // N_CORES      # 2 local batches
D_DEC = N * 3          # 6144
DEC_SH = D_DEC // N_CORES  # 768 decoder columns per core
KDEC = D_DEC // 128    # 48 decoder k tiles
ALPHA = 0.01


# ---------------------------------------------------------------------------
# Workaround: this walrus build caps sync-waits per control instruction very
# low, so TileContext's tail drain (which waits on every proc's semaphore)
# fails codegen. Split the global-clock waits into one single-wait
# EventSemaphore each, then emit a bare Drain.
def _patched_drain_and_barrier(self, tick_clock, wait_clock):
    nc = self.nc
    num_to_handle = {h.num: h for h in self.sems.allocated().values()}
    probe = nc.sync.nop(nofuse=True)
    wait_clock.add_sem_waits(probe.ins, ScopedClock({None: tick_clock.global_clock}))
    waits = list(probe.ins.sync_info.on_wait)
    probe.ins.sync_info = bass_rust.SyncInfo(on_wait=[], on_update=[])
    for w in waits:
        h = num_to_handle.get(w.id)
        if h is None:
            raise RuntimeError(f"no sem handle for {w.id} ({w.ant_name})")
        nc.sync.wait_ge(h, w.wait_value)
    nc.sync.drain()
    nc.all_engine_barrier()
    assert self.sems is not None
    popped = nc._tile_sem_poison_stack.pop()
    assert popped is self._sem_poison
    nc.clear_and_free_semaphores(list(self.sems.allocated().values()))
    nc.all_engine_barrier()


tile.TileContext._drain_and_barrier = _patched_drain_and_barrier


def _split_multi_waits(nc, max_waits=1):
    """This walrus build rejects instructions carrying more than one sync
    wait. Hoist extra waits into standalone EventSemaphore instructions
    placed immediately before the instruction on the same engine queue."""
    ctr = 0
    for fn in nc.m.functions:
        for bb in fn.blocks:
            insts = bb.instructions
            new = []
            changed = False
            for inst in insts:
                si = inst.sync_info
                waits = list(si.on_wait) if si is not None else []
                if len(waits) > max_waits:
                    changed = True
                    for w in waits[:-max_waits]:
                        ev = bass_rust.InstEventSemaphore(
                            name=f"splitw_{ctr}", ins=[], outs=[]
                        )
                        ctr += 1
                        ev.engine = inst.engine
                        ev.sync_info = bass_rust.SyncInfo(
                            on_wait=[w], on_update=[]
                        )
                        new.append(ev)
                    inst.sync_info = bass_rust.SyncInfo(
                        on_wait=waits[-max_waits:], on_update=list(si.on_update)
                    )
                new.append(inst)
            if changed:
                bb.instructions = new


# ---------------------------------------------------------------------------
def _build_program() -> bass.Bass:
    nc = bass.Bass()

    xt = nc.declare_dram_parameter("xt", [BL, NT, 128, CT, 128], BF16, isOutput=False)
    p2t = nc.declare_dram_parameter("p2t", [NT, 128, N], BF16, isOutput=False)
    w01 = nc.declare_dram_parameter("w01", [CT, 128, 512], BF16, isOutput=False)
    w23 = nc.declare_dram_parameter("w23", [4, 128, 256], BF16, isOutput=False)
    w45 = nc.declare_dram_parameter("w45", [2, 128, 3], BF16, isOutput=False)
    exf_a = nc.declare_dram_parameter("exf_a", [2, 512], BF16, isOutput=False)
    exf_b = nc.declare_dram_parameter("exf_b", [2, 256], BF16, isOutput=False)
    exd = nc.declare_dram_parameter("exd", [2, N], BF16, isOutput=False)
    exc = nc.declare_dram_parameter("exc", [2, 2 * 3], BF16, isOutput=False)
    wd = nc.declare_dram_parameter("wd", [KDEC, 128, DEC_SH], BF16, isOutput=False)
    bd = nc.declare_dram_parameter("bd", [1, DEC_SH], BF16, isOutput=False)
    ones16 = nc.declare_dram_parameter("ones16", [1, B], BF16, isOutput=False)
    id16 = nc.declare_dram_parameter("id16", [B, B], BF16, isOutput=False)
    y_out = nc.declare_dram_parameter("y", [B, DEC_SH], F32, isOutput=True)

    cc_in = nc.dram_tensor("cc_in", [BL, D_DEC], BF16)
    cc_out = nc.dram_tensor("cc_out", [B, D_DEC], BF16, addr_space="Shared")

    with tile.TileContext(nc) as tc:
        with (
            tc.tile_pool(name="const", bufs=1) as constp,
            tc.tile_pool(name="xsl", bufs=3) as xpool,
            tc.tile_pool(name="tch", bufs=2 * NT) as tpool,
            tc.tile_pool(name="hch", bufs=2 * 4) as hpool,
            tc.tile_pool(name="t3p", bufs=NT) as t3pool,
            tc.tile_pool(name="lk", bufs=3) as lkpool,
            tc.tile_pool(name="h5p", bufs=1) as h5pool,
            tc.tile_pool(name="ftp", bufs=KDEC) as ftpool,
            tc.tile_pool(name="wdp", bufs=6) as wdpool,
            tc.tile_pool(name="ps", bufs=6, space="PSUM") as ps,
            tc.tile_pool(name="psd", bufs=1, space="PSUM") as psd,
        ):
            # ---- constant tiles (DMA order tuned: L0 inputs first) ----
            slab00 = xpool.tile([128, CT, 128], BF16, tag="xsl")
            nc.sync.dma_start(slab00[:], xt[0, 0])
            w01_sb = constp.tile([128, CT, 512], BF16, tag="w01")
            for cc4 in range(3):
                nc.sync.dma_start(
                    w01_sb[:, cc4 * 4 : (cc4 + 1) * 4, :],
                    w01[cc4 * 4 : (cc4 + 1) * 4].rearrange("c p f -> p c f"),
                )

            # ---- L0: t = x @ (W0 W1), node-major [n,512] bf16 ----
            t_tiles = [[None] * NT for _ in range(BL)]
            for b in range(BL):
                for ni in range(NT):
                    if b == 0 and ni == 0:
                        slab = slab00
                    else:
                        slab = xpool.tile([128, CT, 128], BF16, tag="xsl")
                        nc.sync.dma_start(slab[:], xt[b, ni])
                    pt = ps.tile([128, 512], F32, tag="ps")
                    for ci in range(CT):
                        nc.tensor.matmul(
                            pt[:],
                            slab[:, ci, :],
                            w01_sb[:, ci, :],
                            start=(ci == 0),
                            stop=(ci == CT - 1),
                        )
                    tt = tpool.tile([128, 512], BF16, tag="tch")
                    nc.vector.tensor_copy(tt[:], pt[:])
                    t_tiles[b][ni] = tt

            # ---- remaining constants (DMA'd behind L0's x slabs) ----
            p2t_sb = constp.tile([128, NT, N], BF16, tag="p2t")
            for si in range(NT):
                nc.sync.dma_start(p2t_sb[:, si, :], p2t[si])
            w23_sb = constp.tile([128, 4, 256], BF16, tag="w23")
            nc.sync.dma_start(w23_sb[:], w23[:].rearrange("c p f -> p c f"))
            w45_sb = constp.tile([128, 2, 3], BF16, tag="w45")
            nc.sync.dma_start(w45_sb[:], w45[:].rearrange("c p f -> p c f"))
            exf_a_sb = constp.tile([2, 512], BF16, tag="exfa")
            nc.sync.dma_start(exf_a_sb[:], exf_a[:])
            exf_b_sb = constp.tile([2, 256], BF16, tag="exfb")
            nc.sync.dma_start(exf_b_sb[:], exf_b[:])
            exd_sb = constp.tile([2, N], BF16, tag="exd")
            nc.sync.dma_start(exd_sb[:], exd[:])
            exc_sb = constp.tile([2, 6], BF16, tag="exc")
            nc.sync.dma_start(exc_sb[:], exc[:])
            bd_sb = constp.tile([1, DEC_SH], BF16, tag="bd")
            nc.sync.dma_start(bd_sb[:], bd[:])
            ones16_sb = constp.tile([1, B], BF16, tag="ones16")
            nc.sync.dma_start(ones16_sb[:], ones16[:])
            id16_sb = constp.tile([B, B], BF16, tag="id16")
            nc.sync.dma_start(id16_sb[:], id16[:])

            # ---- decoder weight prefetch: 8 tiles resident from t=0; the
            # other 40 reuse the t/t2 chain slots (tag "tch"), whose last
            # readers finish by stage B — so their DMAs run during stage
            # B/C and the collective window instead of the decoder tail ----
            wd_tiles = []
            for ki in range(6):
                wt = wdpool.tile([128, DEC_SH], BF16, tag="wdp")
                nc.sync.dma_start(wt[:], wd[ki])
                wd_tiles.append(wt)

            # ---- stage A: z1T[f,d] = sum_s t[s,f] P2T[s,d] + bias; leaky ----
            h1_tiles = [[None] * 4 for _ in range(BL)]
            for b in range(BL):
                for fj in range(4):
                    h1 = hpool.tile([128, N], BF16, tag="hch")
                    h1_tiles[b][fj] = h1
                    for dc in range(4):
                        pz = ps.tile([128, 512], F32, tag="ps")
                        for si in range(NT):
                            nc.tensor.matmul(
                                pz[:],
                                t_tiles[b][si][:, fj * 128 : (fj + 1) * 128],
                                p2t_sb[:, si, dc * 512 : (dc + 1) * 512],
                                start=(si == 0),
                                stop=False,
                            )
                        nc.tensor.matmul(
                            pz[:],
                            exf_a_sb[:, fj * 128 : (fj + 1) * 128],
                            exd_sb[:, dc * 512 : (dc + 1) * 512],
                            start=False,
                            stop=True,
                        )
                        s = lkpool.tile([128, 512], F32, tag="lk")
                        nc.scalar.mul(out=s[:], in_=pz[:], mul=ALPHA)
                        nc.vector.tensor_tensor(
                            h1[:, dc * 512 : (dc + 1) * 512],
                            pz[:],
                            s[:],
                            mybir.AluOpType.max,
                        )

            # ---- W23: t2 = h1 @ (W2 W3), node-major [n,256] ----
            t2_tiles = [[None] * NT for _ in range(BL)]
            for b in range(BL):
                for ni in range(NT):
                    pt = ps.tile([128, 512], F32, tag="ps")
                    for cj in range(4):
                        nc.tensor.matmul(
                            pt[:, 0:256],
                            h1_tiles[b][cj][:, ni * 128 : (ni + 1) * 128],
                            w23_sb[:, cj, :],
                            start=(cj == 0),
                            stop=(cj == 3),
                        )
                    tt = tpool.tile([128, 512], BF16, tag="tch")
                    nc.vector.tensor_copy(tt[:, 0:256], pt[:, 0:256])
                    t2_tiles[b][ni] = tt

            # streamed decoder weights: reuse freed t/t2 slots (see above)
            for ki in range(6, KDEC):
                wt = tpool.tile([128, DEC_SH], BF16, tag="tch")
                nc.sync.dma_start(wt[:], wd[ki])
                wd_tiles.append(wt)

            # ---- stage B ----
            h3_tiles = [[None] * 2 for _ in range(BL)]
            for b in range(BL):
                for fj in range(2):
                    h3 = hpool.tile([128, N], BF16, tag="hch")
                    h3_tiles[b][fj] = h3
                    for dc in range(4):
                        pz = ps.tile([128, 512], F32, tag="ps")
                        for si in range(NT):
                            nc.tensor.matmul(
                                pz[:],
                                t2_tiles[b][si][:, fj * 128 : (fj + 1) * 128],
                                p2t_sb[:, si, dc * 512 : (dc + 1) * 512],
                                start=(si == 0),
                                stop=False,
                            )
                        nc.tensor.matmul(
                            pz[:],
                            exf_b_sb[:, fj * 128 : (fj + 1) * 128],
                            exd_sb[:, dc * 512 : (dc + 1) * 512],
                            start=False,
                            stop=True,
                        )
                        s = lkpool.tile([128, 512], F32, tag="lk")
                        nc.scalar.mul(out=s[:], in_=pz[:], mul=ALPHA)
                        nc.vector.tensor_tensor(
                            h3[:, dc * 512 : (dc + 1) * 512],
                            pz[:],
                            s[:],
                            mybir.AluOpType.max,
                        )

            # ---- W45: t3 = h3 @ (W4 W5), node-major [n,3], both batches ----
            t3_tiles = [None] * NT
            for ni in range(NT):
                t3t = t3pool.tile([128, 6], BF16, tag="t3p")
                t3_tiles[ni] = t3t
            for b in range(BL):
                for ni in range(NT):
                    pt = ps.tile([128, 512], F32, tag="ps")
                    for cj in range(2):
                        nc.tensor.matmul(
                            pt[:, 0:3],
                            h3_tiles[b][cj][:, ni * 128 : (ni + 1) * 128],
                            w45_sb[:, cj, :],
                            start=(cj == 0),
                            stop=(cj == 1),
                        )
                    nc.vector.tensor_copy(
                        t3_tiles[ni][:, b * 3 : (b + 1) * 3], pt[:, 0:3]
                    )

            # ---- stage C: z5 = P2 @ t3 + bias; leaky -> h5 [d, 6] ----
            cc_in_r = cc_in[:].rearrange("b (d p c) -> b p d c", d=NT, p=128, c=3)
            h5_all = h5pool.tile([128, NT, 6], BF16, tag="h5p")
            for di in range(NT):
                pz = ps.tile([128, 512], F32, tag="ps")
                for si in range(NT):
                    nc.tensor.matmul(
                        pz[:, 0:6],
                        p2t_sb[:, si, di * 128 : (di + 1) * 128],
                        t3_tiles[si][:],
                        start=(si == 0),
                        stop=False,
                    )
                nc.tensor.matmul(
                    pz[:, 0:6],
                    exd_sb[:, di * 128 : (di + 1) * 128],
                    exc_sb[:],
                    start=False,
                    stop=True,
                )
                s = lkpool.tile([128, 512], F32, tag="lk")
                nc.scalar.mul(out=s[:, 0:6], in_=pz[:, 0:6], mul=ALPHA)
                nc.vector.tensor_tensor(
                    h5_all[:, di, :], pz[:, 0:6], s[:, 0:6], mybir.AluOpType.max
                )
            for b in range(BL):
                nc.sync.dma_start(
                    cc_in_r[b], h5_all[:, :, b * 3 : (b + 1) * 3]
                )

            # ---- AllGather feat across the 8 cores ----
            nc.gpsimd.collective_compute(
                "AllGather",
                mybir.AluOpType.bypass,
                replica_groups=[list(range(N_CORES))],
                ins=[cc_in[:]],
                outs=[cc_out[:]],
            )

            # ---- decoder: y = tanh(feat @ Wd_shard + bd_shard) * 0.1 ----
            feat_sb = constp.tile([B, D_DEC], BF16, tag="feat")
            nc.sync.dma_start(feat_sb[:], cc_out[:])

            featT = [None] * KDEC
            for ki in range(KDEC):
                ptr = ps.tile([128, B], BF16, tag="ps")
                nc.tensor.transpose(
                    ptr[:], feat_sb[:, ki * 128 : (ki + 1) * 128], id16_sb[:]
                )
                ft = ftpool.tile([128, B], BF16, tag="ftp")
                nc.vector.tensor_copy(ft[:], ptr[:])
                featT[ki] = ft

            pd = psd.tile([B, 1024], F32, tag="psd")
            for ki in range(KDEC):
                wt = wd_tiles[ki]
                nc.tensor.matmul(
                    pd[:, 0:512], featT[ki][:], wt[:, 0:512],
                    start=(ki == 0), stop=False,
                )
                nc.tensor.matmul(
                    pd[:, 512:768], featT[ki][:], wt[:, 512:768],
                    start=(ki == 0), stop=False,
                )
            nc.tensor.matmul(
                pd[:, 0:512], ones16_sb[:], bd_sb[:, 0:512], start=False, stop=True
            )
            nc.tensor.matmul(
                pd[:, 512:768], ones16_sb[:], bd_sb[:, 512:768],
                start=False, stop=True,
            )

            y_sb = constp.tile([B, DEC_SH], F32, tag="ysb")
            nc.scalar.activation(
                y_sb[:, 0:512], pd[:, 0:512], mybir.ActivationFunctionType.Tanh
            )
            nc.scalar.activation(
                y_sb[:, 512:768], pd[:, 512:768], mybir.ActivationFunctionType.Tanh
            )
            nc.scalar.mul(out=y_sb[:], in_=y_sb[:], mul=0.1)
            nc.sync.dma_start(y_out[:], y_sb[:])

    _split_multi_waits(nc)
    return nc


# ---------------------------------------------------------------------------
def _host_precompute(x, edges, Ws, bs, Wd, bd_np):
    """Build all per-core device inputs from the raw problem inputs."""
    edges = np.asarray(edges)
    src = edges[0].astype(np.int64)
    dst = edges[1].astype(np.int64)

    deg = np.bincount(dst, minlength=N).astype(np.float64) + 1.0
    isd = 1.0 / np.sqrt(deg)
    idg = 1.0 / deg

    P = np.zeros((N, N), np.float64)
    np.add.at(P, (dst, src), isd[src] * isd[dst])
    P[np.arange(N), np.arange(N)] += idg
    P2 = P @ P
    r = P.sum(axis=1)

    W0, W1, W2, W3, W4, W5 = [np.asarray(w, np.float64) for w in Ws]
    b0, b1, b2, b3, b4, b5 = [np.asarray(b, np.float64) for b in bs]
    W01 = W0 @ W1          # [1475, 512]
    W23 = W2 @ W3          # [512, 256]
    W45 = W4 @ W5          # [256, 3]
    a1 = b0 @ W1           # [512]
    a3 = b2 @ W3           # [256]
    a5 = b4 @ W5           # [3]

    # p2t[si, p, d] = P2[d, si*128+p]  (P2^T tiled on the contraction dim)
    p2t = np.ascontiguousarray(
        P2.T.reshape(NT, 128, N).astype(NPBF16)
    )

    w01_pad = np.zeros((C_PAD, 512), np.float64)
    w01_pad[:C_IN] = W01
    w01_t = np.ascontiguousarray(w01_pad.reshape(CT, 128, 512).astype(NPBF16))
    w23_t = np.ascontiguousarray(W23.reshape(4, 128, 256).astype(NPBF16))
    w45_t = np.ascontiguousarray(W45.reshape(2, 128, 3).astype(NPBF16))

    exf_a_np = np.stack([a1, b1]).astype(NPBF16)         # [2, 512]
    exf_b_np = np.stack([a3, b3]).astype(NPBF16)         # [2, 256]
    exd_np = np.stack([r, np.ones(N)]).astype(NPBF16)    # [2, N]
    exc_np = np.stack(
        [np.concatenate([a5, a5]), np.concatenate([b5, b5])]
    ).astype(NPBF16)                                     # [2, 6]

    # xt[b, ni, p_n, ci, j_c]?? -> layout [BL, NT, 128, CT, 128] with
    # xt[b, ni, p, ci, j] = x[b, ni*128 + j, ci*128 + p]   (p = channel-in-tile,
    # j = node-in-tile): lhsT tile = [channel partition, node free].
    x_np = np.asarray(x, np.float32)
    x_pad = np.zeros((B, N, C_PAD), np.float32)
    x_pad[:, :, :C_IN] = x_np
    # [B, NT, j, CT, p] -> transpose to [B, NT, p, CT, j]
    xt_all = np.ascontiguousarray(
        x_pad.reshape(B, NT, 128, CT, 128).transpose(0, 1, 4, 3, 2).astype(NPBF16)
    )

    Wd_np = np.asarray(Wd, np.float32)
    bd_full = np.asarray(bd_np, np.float32)

    ones16_np = np.ones((1, B), NPBF16)
    id16_np = np.eye(B).astype(NPBF16)

    shared = {
        "p2t": p2t,
        "w01": w01_t,
        "w23": w23_t,
        "w45": w45_t,
        "exf_a": exf_a_np,
        "exf_b": exf_b_np,
        "exd": exd_np,
        "exc": exc_np,
        "ones16": ones16_np,
        "id16": id16_np,
    }
    in_maps = []
    for c in range(N_CORES):
        wd_c = np.ascontiguousarray(
            Wd_np[:, c * DEC_SH : (c + 1) * DEC_SH]
            .reshape(KDEC, 128, DEC_SH)
            .astype(NPBF16)
        )
        bd_c = np.ascontiguousarray(
            bd_full[c * DEC_SH : (c + 1) * DEC_SH].reshape(1, DEC_SH).astype(NPBF16)
        )
        m = dict(shared)
        m["xt"] = xt_all[c * BL : (c + 1) * BL]
        m["wd"] = wd_c
        m["bd"] = bd_c
        in_maps.append(m)
    return in_maps


_NC_CACHE = {}


def kernel(**inputs) -> np.ndarray:
    x = inputs["x"]
    edges = inputs["edges"]
    Ws = [inputs[f"W{i}"] for i in range(6)]
    bs = [inputs[f"b{i}"] for i in range(6)]
    Wd = inputs["Wd"]
    bd_np = inputs["bd"]

    in_maps = _host_precompute(x, edges, Ws, bs, Wd, bd_np)

    if "nc" not in _NC_CACHE:
        _NC_CACHE["nc"] = _build_program()
    nc = _NC_CACHE["nc"]

    res = run_bass_kernel_spmd(nc, in_maps, list(range(N_CORES)))

    out = np.zeros((B, D_DEC), np.float32)
    for c in range(N_CORES):
        out[:, c * DEC_SH : (c + 1) * DEC_SH] = res.results[c]["y"]
    return out.reshape(B, N, 3)
